# revision 19
# baseline (speedup 1.0000x reference)
"""MoE gate kernel for Trainium2 (8 NeuronCores, SPMD data-parallel over tokens).

reference:
    scores = sigmoid(x @ W.T)            # [T, E] fp32
    biased = scores + bias
    inds   = top_k(-biased, 8).indices   # 8 smallest biased, ascending biased
    sel    = scores[inds] / sum * 2.5

Numerics: logits are computed to ~1.3e-5 abs error (vs logit std 1.28) with
2.0 fp16-matmul-equivalents of PE work instead of the naive 3:
    x  = a + b,   a = fp16(x),  b = x - a
    Wt = c + d,   c = fp16(Wt), d = Wt - c
    x@Wt ~= a@c  +  a@d + b@c      (b@d ~ 2^-22 relative, dropped)
  - main term:   (a*2^7) @ (c*2^7)            fp16 matmul,   scale 2^14
  - corrections: e4m3(a) @ e4m3(d*2^14)       fp8 DoubleRow (2x rate)
                 e4m3(b*2^11) @ e4m3(c*2^3)   fp8 DoubleRow
  All terms accumulate into ONE PSUM bank at common scale 2^14; the ACT
  sigmoid applies scale=2^-14 on read. HW fp16/fp8 matmul numerics verified
  bit-close to the numpy/ml_dtypes simulation (6/16384 tokens flip an index,
  combined rel err 0.005 << 0.02 gate).

Device flow per 128-token tile: 32 fp16 matmuls + 32 fp8 DoubleRow matmuls
(contraction 256/instr) -> sigmoid(scale) -> DVE keys = -bias - scores ->
DVE max8/max_index -> 8 idx + 8 key words per token. The host recovers the
selected original scores as s_j = -key_j - bias[idx_j] (exact algebra,
O(T*K) work) and normalizes: sel = 2.5 * s / s.sum().
"""

import sys

sys.path.insert(0, "/opt/trn_rl_repo")

import numpy as np
import ml_dtypes

import concourse.bacc as bacc
import concourse.mybir as mybir
import concourse.tile as tile
from concourse import bass_utils

T, H, E, K = 16384, 4096, 256, 8
N_CORES = 8
TS = T // N_CORES          # tokens per core
TCHUNK = 128               # tokens per PE tile (PSUM partition dim)
NT = TS // TCHUNK          # token tiles per core
F = H // 128               # h-slices per partition block
ROUTED_SCALING = 2.5
E4 = ml_dtypes.float8_e4m3

f32 = mybir.dt.float32
f16 = mybir.dt.float16
f8 = mybir.dt.float8e4
u32 = mybir.dt.uint32
Alu = mybir.AluOpType
Act = mybir.ActivationFunctionType
DR = mybir.MatmulPerfMode.DoubleRow
DRSW = mybir.MatmulPerfMode.DoubleRowSwInterleave


def build_nc(nt=NT):
    """Build the SPMD Bass program for one core handling nt*TCHUNK tokens."""
    nc = bacc.Bacc("TRN2", target_bir_lowering=False, debug=False,
                   num_devices=N_CORES)

    # pre-tiled on host: [it, p, f*TCHUNK + t] = arr[it*TCHUNK + t, 32p + f]
    a16_d = nc.dram_tensor("a16", [nt, 128, F * TCHUNK], f16,
                           kind="ExternalInput")
    # fp8 x payload: [it, p, f, t] = e4m3(b*2^11); e4m3(a) is cast on-device
    x8_d = nc.dram_tensor("x8", [nt, 128, F * TCHUNK], f8,
                          kind="ExternalInput")
    c16_d = nc.dram_tensor("c16", [H, E], f16, kind="ExternalInput")
    # fp8 w payload: [p, f, e] = e4m3(d*2^14); e4m3(c*2^3) is cast on-device
    w8_d = nc.dram_tensor("w8", [128, F, E], f8, kind="ExternalInput")
    nbias_d = nc.dram_tensor("nbias", [128, E], f32, kind="ExternalInput")
    out_d = nc.dram_tensor("out", [128, nt * 2 * K], u32, kind="ExternalOutput")

    with tile.TileContext(nc) as tc:
        with (
            tc.tile_pool(name="const", bufs=1) as cpool,
            tc.tile_pool(name="xa", bufs=3) as xapool,
            tc.tile_pool(name="x8p", bufs=2) as x8pool,
            tc.tile_pool(name="a8p", bufs=6) as a8pool,
            tc.tile_pool(name="sc", bufs=4) as spool,
            tc.tile_pool(name="ps", bufs=5, space="PSUM") as ppool,
        ):
            # nbias first on sync (tiny; needed by the first DVE subtract)
            nb = cpool.tile([128, E], f32, tag="nb")
            nc.sync.dma_start(nb[:], nbias_d.ap())
            # weights ride the scalar queue exclusively: all of c16 (needed
            # within ~3.5us of the first matmul), then w8 (needed only when
            # the first deferred DR block runs). a16 rides sync; x8 rides
            # the GPSIMD SWDGE queue, so the three streams never queue
            # behind each other.
            c16_src = c16_d.ap().rearrange("(p f) e -> p f e", f=F)
            C16_CH = [(0, 4), (4, 8), (8, 12), (12, 16), (16, 20), (20, 24),
                      (24, 28), (28, 32)]
            c16_t = []
            for ci, (f0, f1) in enumerate(C16_CH):
                t = cpool.tile([128, f1 - f0, E], f16, tag=f"c16{ci}")
                nc.scalar.dma_start(t[:], c16_src[:, f0:f1, :])
                c16_t.append((f0, t))

            def c16_ap(f):
                for (f0, t) in reversed(c16_t):
                    if f >= f0:
                        return t[:, f - f0, :]
            d8_c = []
            for c in range(2):
                ks = slice(c * 16, (c + 1) * 16)
                t8 = cpool.tile([128, 16, E], f8, tag=f"d8{c}")
                nc.scalar.dma_start(t8[:], w8_d.ap()[:, ks, :])
                d8_c.append(t8)
            # c8s = e4m3(c*2^3) cast on-device from the 2^7-scaled c16 chunks
            c8t = cpool.tile([128, F, E], f8, tag="c8s")
            for ci, (f0, f1) in enumerate(C16_CH):
                t = c16_t[ci][1]
                dst = c8t[:, f0:f1, :]
                if ci < 4:
                    nc.scalar.activation(dst, t[:], Act.Copy,
                                         scale=2.0 ** -4)
                else:
                    nc.vector.tensor_scalar(dst, t[:],
                                            2.0 ** -4, None, Alu.mult)
            obuf = cpool.tile([128, nt * 2 * K], u32, tag="obuf")

            # PE warm-up: hold the p-state clock at full speed until the
            # startup payload lands (~16.5us). Single-bank tile in the main
            # PSUM pool (bufs=1) so no nested pool context serializes it.
            wz = cpool.tile([128, 128], f16, tag="wz")
            nc.gpsimd.memset(wz[:], 0)
            wps = ppool.tile([128, 128], f32, tag="warm", bufs=1)
            for _ in range(150):
                nc.tensor.matmul(wps[:], wz[:], wz[:])

            FH = F // 4

            def a16_ap(ach, f):
                for (f0, f1, t) in reversed(ach):
                    if f >= f0:
                        return t[:, f - f0, :]

            D = 3      # DR/post blocks run D tiles behind the fp16 block so
                       # the weight payload is never on the PE critical
                       # path at startup
            accs = {}
            x8ts = {}
            a8ts = {}

            def dr_post(jt):
                acc, b8t, a8t = accs.pop(jt), x8ts.pop(jt), a8ts.pop(jt)
                for fd in range(F):
                    if fd < F // 2:
                        stat = a8t[:, 2 * fd:2 * fd + 2, :]
                        mov = d8_c[fd // 8][:, (2 * fd) % 16:
                                            (2 * fd) % 16 + 2, :]
                        pm = DR
                    else:
                        stat = b8t[:, 2 * fd - F:2 * fd - F + 2, :]
                        mov = c8t[:, 2 * fd - F:2 * fd - F + 2, :]
                        pm = DRSW
                    nc.tensor.matmul(acc[:], stat, mov,
                                     start=False, stop=(fd == F - 1),
                                     perf_mode=pm)
                scores = spool.tile([128, E], f32, tag="scores")
                nc.scalar.activation(scores[:], acc[:], Act.Sigmoid,
                                     scale=2.0 ** -14)
                negb = spool.tile([128, E], f32, tag="negb")
                nc.vector.tensor_tensor(negb[:], nb[:], scores[:], Alu.subtract)
                m8 = obuf[:, jt * 2 * K + K: (jt + 1) * 2 * K].bitcast(f32)
                nc.vector.max(m8, negb[:])
                nc.vector.max_index(obuf[:, jt * 2 * K: jt * 2 * K + K],
                                    m8, negb[:])

            for it in range(nt):
                a_src = a16_d.ap()[it].rearrange("p (f t) -> p f t", f=F)
                x8_src = x8_d.ap()[it].rearrange("p (k t) -> p k t", k=F)
                ach = []
                for c in range(4):
                    f0, f1 = c * FH, (c + 1) * FH
                    t = xapool.tile([128, FH, TCHUNK], f16, tag=f"a{c}")
                    nc.sync.dma_start(t[:], a_src[:, f0:f1, :])
                    ach.append((f0, f1, t))
                x8t = x8pool.tile([128, F, TCHUNK], f8, tag="x8")
                nc.gpsimd.dma_start(x8t[:], x8_src[:])
                x8ts[it] = x8t

                acc = ppool.tile([128, E], f32, tag="acc")
                accs[it] = acc
                for f in range(F):
                    nc.tensor.matmul(acc[:], a16_ap(ach, f),
                                     c16_ap(f),
                                     start=(f == 0), stop=False)
                # cast e4m3(a) from the scaled fp16 tiles (scale 2^-7),
                # split across the idle ACT and DVE engines
                a8t = a8pool.tile([128, F, TCHUNK], f8, tag="a8")
                for c, (f0, f1, t) in enumerate(ach):
                    dst = a8t[:, f0:f1, :]
                    if c < len(ach) // 2:
                        nc.scalar.activation(dst, t[:], Act.Copy,
                                             scale=2.0 ** -7)
                    else:
                        nc.vector.tensor_scalar(dst, t[:],
                                                2.0 ** -7, None, Alu.mult)
                a8ts[it] = a8t
                if it >= D:
                    dr_post(it - D)
            for jt in range(nt - D, nt):
                dr_post(jt)
                if jt == nt - D:
                    nc.sync.dma_start(out_d.ap()[:, :nt * K],
                                      obuf[:, :nt * K])

            nc.sync.dma_start(out_d.ap()[:, nt * K:], obuf[:, nt * K:])

    nc.compile()
    return nc


def host_prep(x, weight, e_score_correction_bias):
    """Split x/W into fp16 + scaled-fp8 payloads; pretile x per core."""
    x = np.asarray(x, dtype=np.float32)
    w = np.asarray(weight, dtype=np.float32)
    b = np.asarray(e_score_correction_bias, dtype=np.float32)

    a = x.astype(np.float16)
    bres = x - a.astype(np.float32)
    a16s = (a.astype(np.float32) * 128.0).astype(np.float16)  # exact *2^7
    b8s = (bres * 2.0 ** 11).astype(E4)

    wt = np.ascontiguousarray(w.T)     # [H, E]
    c = wt.astype(np.float16)
    d = wt - c.astype(np.float32)
    c16s = (c.astype(np.float32) * 128.0).astype(np.float16)  # exact *2^7
    d8s = (d * 2.0 ** 14).astype(E4)

    def pretile2(arr):  # [TS, H] u16view -> [NT, 128, F*TCHUNK]
        arr = arr.reshape(NT, TCHUNK, 128, F).transpose(0, 2, 3, 1)
        return np.ascontiguousarray(arr).reshape(NT, 128, F * TCHUNK)

    def swinterleave(p):
        # [NT, 128, F*T] -> per k-tile-pair block: A127 B127 A126 B126 ... B0
        # (pair-interleaved columns stored in reverse order, as the PE's
        # DoubleRowSwInterleave mode expects for the stationary operand)
        p4 = p.reshape(NT, 128, F // 2, 2, TCHUNK)[..., ::-1]
        return np.ascontiguousarray(
            p4.transpose(0, 1, 2, 4, 3)).reshape(NT, 128, F * TCHUNK)

    w8 = np.ascontiguousarray(d8s.reshape(128, F, E))

    nbias = np.ascontiguousarray(np.broadcast_to(-b, (128, E)))

    in_maps = []
    for cid in range(N_CORES):
        sl = slice(cid * TS, (cid + 1) * TS)
        in_maps.append({
            "a16": pretile2(a16s[sl].view(np.uint16)).view(np.float16),
            "x8": swinterleave(pretile2(b8s[sl].view(np.uint8))).view(E4),
            "c16": c16s,
            "w8": w8,
            "nbias": nbias,
        })
    return in_maps


def unpack(out_cores, b):
    """list of [128, NT*16] u32 -> (inds int32 [T, 8], sel float32 [T, 8])."""
    inds = np.empty((T, K), dtype=np.int32)
    sel = np.empty((T, K), dtype=np.float32)
    for c, o in enumerate(out_cores):
        o = o.reshape(128, NT, 2 * K).transpose(1, 0, 2)  # [it, p, 16]
        o = np.ascontiguousarray(o).reshape(TS, 2 * K)
        ii = o[:, :K].astype(np.int32)
        keys = o[:, K:].view(np.float32)
        s = (-keys - b[ii]).astype(np.float32)   # selected original scores
        sv = s / s.sum(axis=-1, keepdims=True) * np.float32(ROUTED_SCALING)
        inds[c * TS:(c + 1) * TS] = ii
        sel[c * TS:(c + 1) * TS] = sv
    return inds, sel


_NC_CACHE = {}


def _get_nc():
    if "nc" not in _NC_CACHE:
        _NC_CACHE["nc"] = build_nc()
    return _NC_CACHE["nc"]


def kernel(x, weight, e_score_correction_bias, _trace=False):
    b = np.asarray(e_score_correction_bias, dtype=np.float32)
    in_maps = host_prep(x, weight, e_score_correction_bias)
    nc = _get_nc()
    res = bass_utils.run_bass_kernel_spmd(
        nc, in_maps, list(range(N_CORES)), trace=_trace)
    inds, sel = unpack([res.results[c]["out"] for c in range(N_CORES)], b)
    if _trace:
        kernel.last_results = res
    return inds, sel
